# revision 22
# baseline (speedup 1.0000x reference)
"""Trainium2 Bass kernel for nn_ExtendedSelfAttention (B=4, S=2048, D=4096, H=1).

With n_heads=1 the softmax is over a size-1 axis, so attention weights are
exactly 1.0 and the module reduces to:

    out = (value @ Wv.T + bv) @ Wo.T + bo
        = value @ (Wo @ Wv).T + (Wo @ bv + bo)

(query/key/Wq/Wk never affect the output.) Since there are 8192 tokens but
only 4096 features, composing the weights first cuts total FLOPs by 25%:
computing Wc^T = (Wo @ Wv)^T costs one 4096^3 GEMM (sharded 8 ways), after
which only ONE token GEMM is needed instead of two.

Sharding (no collectives):
  phase A: core c computes Wc^T[:, c*512:(c+1)*512]   (1024 matmuls)
           lhsT = Wv[f-tile, k-block] (natural layout), rhs = Wo^T slice
  phase B: core c computes out[:, c*512:(c+1)*512] for ALL 8192 tokens
           lhsT = x^T tiles, rhs = Wc^T slice (SBUF-resident)  (2048 matmuls)
Output is column-sharded; the host concatenates. The fused bias
bias2 = Wo @ bv + bo is computed exactly on the host and added in phase B.

Compute dtype bf16 (host-cast), fp32 PSUM accumulation, fp32 output.
"""

import numpy as np

B, S, D = 4, 2048, 4096
N_CORES = 8
TOK = B * S           # 8192 tokens
P = 128
KO = D // P           # 32 contraction tiles
GBLK = D // N_CORES   # 512 output columns per core
TT = TOK // P         # 64 token tiles

_CACHED = {}


def _build_nc():
    import concourse.bass as bass  # noqa: F401  (registers engine builders)
    import concourse.tile as tile
    from concourse import bacc, mybir

    bf16 = mybir.dt.bfloat16
    f32 = mybir.dt.float32

    nc = bacc.Bacc("TRN2", target_bir_lowering=False, debug=False,
                   num_devices=N_CORES)

    # wv[m, p, fo, c2] = Wv[fo*128+p, m*128+c2]   (lhsT tiles for phase A)
    wv = nc.declare_dram_parameter("wv", [KO, P, KO, P], bf16, isOutput=False)
    # woT[p, fo, g] = Wo[cg0+g, fo*128+p]          (rhs for phase A, per-core)
    woT = nc.declare_dram_parameter("woT", [P, KO, GBLK], bf16, isOutput=False)
    # xt[tt, p, ko, tc] = x[tt*128+tc, ko*128+p]   (lhsT tiles for phase B)
    xt = nc.declare_dram_parameter("xt", [TT, P, KO, P], bf16, isOutput=False)
    b2 = nc.declare_dram_parameter("b2", [P, GBLK], f32, isOutput=False)
    out = nc.declare_dram_parameter("out", [TOK, GBLK], f32, isOutput=True)

    with tile.TileContext(nc) as tc:
        with tc.tile_pool(name="const", bufs=1) as const_pool, \
             tc.tile_pool(name="wot", bufs=1) as wot_pool, \
             tc.tile_pool(name="wct", bufs=1) as wct_pool, \
             tc.tile_pool(name="wvp", bufs=4) as wv_pool, \
             tc.tile_pool(name="xtp", bufs=4) as xt_pool, \
             tc.tile_pool(name="psum", bufs=8, space="PSUM") as psum_pool, \
             tc.tile_pool(name="stage", bufs=4) as stage_pool:
            wot_sb = wot_pool.tile([P, KO, GBLK], bf16)
            wct_sb = wct_pool.tile([P, KO, GBLK], bf16)

            # ---- phase A: Wc^T slice = Wv.T-contracted with Wo^T slice ----
            # Startup: interleave wot 8-ftile chunks (8KB/partition descriptors
            # -> full DMA rate) with the first wv tiles so the first matmul
            # group can start ~5us in and never starves afterwards.
            # startup loads split across two issue queues (sync + gpsimd) so
            # the per-dma_start descriptor-generation latencies overlap
            wv_pre = []
            nc.sync.dma_start(out=wot_sb[:, 0:8, :], in_=woT[:, 0:8, :])
            wv_t = wv_pool.tile([P, KO, P], bf16, tag="wv")
            nc.gpsimd.dma_start(out=wv_t[:], in_=wv[0])
            wv_pre.append(wv_t)
            for g in range(1, 4):
                nc.sync.dma_start(out=wot_sb[:, g * 8:(g + 1) * 8, :],
                                  in_=woT[:, g * 8:(g + 1) * 8, :])
            for m in range(1, 3):
                wv_t = wv_pool.tile([P, KO, P], bf16, tag="wv")
                nc.gpsimd.dma_start(out=wv_t[:], in_=wv[m])
                wv_pre.append(wv_t)

            b2_t = const_pool.tile([P, GBLK], f32)
            nc.sync.dma_start(out=b2_t[:], in_=b2[:])

            for mA in range(KO):
                if mA < 3:
                    wv_t = wv_pre[mA]
                else:
                    wv_t = wv_pool.tile([P, KO, P], bf16, tag="wv")
                    nc.sync.dma_start(out=wv_t[:], in_=wv[mA])
                ps = psum_pool.tile([P, GBLK], f32)
                for fA in range(KO):
                    nc.tensor.matmul(
                        ps[:], wv_t[:, fA, :], wot_sb[:, fA, :],
                        start=(fA == 0), stop=(fA == KO - 1),
                    )
                nc.vector.tensor_copy(wct_sb[:, mA, :], ps[:])

            # ---- phase B: out slice = x @ Wc^T slice (+ bias2) ----
            for tt in range(TT):
                xt_t = xt_pool.tile([P, KO, P], bf16)
                nc.sync.dma_start(out=xt_t[:], in_=xt[tt])
                ps = psum_pool.tile([P, GBLK], f32)
                for k in range(KO):
                    nc.tensor.matmul(
                        ps[:], xt_t[:, k, :], wct_sb[:, k, :],
                        start=(k == 0), stop=(k == KO - 1),
                    )
                st = stage_pool.tile([P, GBLK], f32)
                nc.vector.tensor_add(st[:], ps[:], b2_t[:])
                nc.sync.dma_start(
                    out=out[tt * P:(tt + 1) * P, :], in_=st[:])
    nc.compile()
    return nc


def _get_nc():
    if "nc" not in _CACHED:
        _CACHED["nc"] = _build_nc()
    return _CACHED["nc"]


def _prep_inputs(value, Wv, bv, Wo, bo):
    import ml_dtypes
    bf16 = ml_dtypes.bfloat16

    x = np.asarray(value, np.float32).reshape(TOK, D)
    Wv = np.asarray(Wv, np.float32)
    Wo = np.asarray(Wo, np.float32)
    bv = np.asarray(bv, np.float32)
    bo = np.asarray(bo, np.float32)

    # xt[tt, p, ko, tc] = x[tt*128+tc, ko*128+p]
    xt = np.ascontiguousarray(
        x.reshape(TT, P, KO, P).transpose(0, 3, 2, 1)).astype(bf16)
    # wv_p[m, p, fo, c2] = Wv[fo*128+p, m*128+c2]
    wv_p = np.ascontiguousarray(
        Wv.reshape(KO, P, KO, P).transpose(2, 1, 0, 3)).astype(bf16)
    # woT_full[c][p, fo, g] = Wo[c*GBLK+g, fo*128+p]
    woT_full = Wo.reshape(N_CORES, GBLK, KO, P).transpose(0, 3, 2, 1)

    bias2 = (Wo.astype(np.float64) @ bv.astype(np.float64)
             + bo.astype(np.float64)).astype(np.float32)

    in_maps = []
    for c in range(N_CORES):
        b2_c = np.ascontiguousarray(np.broadcast_to(
            bias2[c * GBLK:(c + 1) * GBLK][None, :], (P, GBLK)))
        in_maps.append({
            "xt": xt,
            "wv": wv_p,
            "woT": np.ascontiguousarray(woT_full[c]).astype(bf16),
            "b2": b2_c,
        })
    return in_maps


def _run(in_maps, trace=False):
    from concourse.bass_utils import run_bass_kernel_spmd
    nc = _get_nc()
    res = run_bass_kernel_spmd(nc, in_maps, list(range(N_CORES)), trace=trace)
    return res


def kernel(**inputs):
    in_maps = _prep_inputs(inputs["value"], inputs["Wv"], inputs["bv"],
                           inputs["Wo"], inputs["bo"])
    res = _run(in_maps, trace=False)
    out = np.empty((TOK, D), np.float32)
    for c in range(N_CORES):
        out[:, c * GBLK:(c + 1) * GBLK] = res.results[c]["out"]
    return out.reshape(B, S, D)


# revision 23
# speedup vs baseline: 1.0103x; 1.0103x over previous
"""Trainium2 Bass kernel for nn_ExtendedSelfAttention (B=4, S=2048, D=4096, H=1).

With n_heads=1 the softmax is over a size-1 axis, so attention weights are
exactly 1.0 and the module reduces to:

    out = (value @ Wv.T + bv) @ Wo.T + bo
        = value @ (Wo @ Wv).T + (Wo @ bv + bo)

(query/key/Wq/Wk never affect the output.) Since there are 8192 tokens but
only 4096 features, composing the weights first cuts total FLOPs by 25%:
computing Wc^T = (Wo @ Wv)^T costs one 4096^3 GEMM (sharded 8 ways), after
which only ONE token GEMM is needed instead of two.

Sharding (no collectives):
  phase A: core c computes Wc^T[:, c*512:(c+1)*512]   (1024 matmuls)
           lhsT = Wv[f-tile, k-block] (natural layout), rhs = Wo^T slice
  phase B: core c computes out[:, c*512:(c+1)*512] for ALL 8192 tokens
           lhsT = x^T tiles, rhs = Wc^T slice (SBUF-resident)  (2048 matmuls)
Output is column-sharded; the host concatenates. The fused bias
bias2 = Wo @ bv + bo is computed exactly on the host and added in phase B.

Compute dtype bf16 (host-cast), fp32 PSUM accumulation, fp32 output.
"""

import numpy as np

B, S, D = 4, 2048, 4096
N_CORES = 8
TOK = B * S           # 8192 tokens
P = 128
KO = D // P           # 32 contraction tiles
GBLK = D // N_CORES   # 512 output columns per core
TT = TOK // P         # 64 token tiles

_CACHED = {}


def _build_nc():
    import concourse.bass as bass  # noqa: F401  (registers engine builders)
    import concourse.tile as tile
    from concourse import bacc, mybir

    bf16 = mybir.dt.bfloat16
    f32 = mybir.dt.float32

    nc = bacc.Bacc("TRN2", target_bir_lowering=False, debug=False,
                   num_devices=N_CORES)

    # wv[m, p, fo, c2] = Wv[fo*128+p, m*128+c2]   (lhsT tiles for phase A)
    wv = nc.declare_dram_parameter("wv", [KO, P, KO, P], bf16, isOutput=False)
    # woT[p, fo, g] = Wo[cg0+g, fo*128+p]          (rhs for phase A, per-core)
    woT = nc.declare_dram_parameter("woT", [P, KO, GBLK], bf16, isOutput=False)
    # xt[tt, p, ko, tc] = x[tt*128+tc, ko*128+p]   (lhsT tiles for phase B)
    xt = nc.declare_dram_parameter("xt", [TT, P, KO, P], bf16, isOutput=False)
    b2 = nc.declare_dram_parameter("b2", [P, GBLK], f32, isOutput=False)
    out = nc.declare_dram_parameter("out", [TOK, GBLK], f32, isOutput=True)

    with tile.TileContext(nc) as tc:
        with tc.tile_pool(name="const", bufs=1) as const_pool, \
             tc.tile_pool(name="wot", bufs=1) as wot_pool, \
             tc.tile_pool(name="wct", bufs=1) as wct_pool, \
             tc.tile_pool(name="wvp", bufs=4) as wv_pool, \
             tc.tile_pool(name="xtp", bufs=4) as xt_pool, \
             tc.tile_pool(name="psum", bufs=8, space="PSUM") as psum_pool, \
             tc.tile_pool(name="stage", bufs=4) as stage_pool:
            wot_sb = wot_pool.tile([P, KO, GBLK], bf16)
            wct_sb = wct_pool.tile([P, KO, GBLK], bf16)

            # ---- phase A: Wc^T slice = Wv.T-contracted with Wo^T slice ----
            # Startup: interleave wot 8-ftile chunks (8KB/partition descriptors
            # -> full DMA rate) with the first wv tiles so the first matmul
            # group can start ~5us in and never starves afterwards.
            wv_pre = []
            nc.sync.dma_start(out=wot_sb[:, 0:8, :], in_=woT[:, 0:8, :])
            wv_t = wv_pool.tile([P, KO, P], bf16, tag="wv")
            nc.sync.dma_start(out=wv_t[:], in_=wv[0])
            wv_pre.append(wv_t)
            for g in range(1, 4):
                nc.sync.dma_start(out=wot_sb[:, g * 8:(g + 1) * 8, :],
                                  in_=woT[:, g * 8:(g + 1) * 8, :])
            for m in range(1, 3):
                wv_t = wv_pool.tile([P, KO, P], bf16, tag="wv")
                nc.sync.dma_start(out=wv_t[:], in_=wv[m])
                wv_pre.append(wv_t)

            b2_t = const_pool.tile([P, GBLK], f32)
            nc.sync.dma_start(out=b2_t[:], in_=b2[:])

            for mA in range(KO):
                if mA < 3:
                    wv_t = wv_pre[mA]
                else:
                    wv_t = wv_pool.tile([P, KO, P], bf16, tag="wv")
                    nc.sync.dma_start(out=wv_t[:], in_=wv[mA])
                ps = psum_pool.tile([P, GBLK], f32)
                for fA in range(KO):
                    nc.tensor.matmul(
                        ps[:], wv_t[:, fA, :], wot_sb[:, fA, :],
                        start=(fA == 0), stop=(fA == KO - 1),
                    )
                nc.vector.tensor_copy(wct_sb[:, mA, :], ps[:])

            # ---- phase B: out slice = x @ Wc^T slice (+ bias2) ----
            for tt in range(TT):
                xt_t = xt_pool.tile([P, KO, P], bf16)
                nc.sync.dma_start(out=xt_t[:], in_=xt[tt])
                ps = psum_pool.tile([P, GBLK], f32)
                for k in range(KO):
                    nc.tensor.matmul(
                        ps[:], xt_t[:, k, :], wct_sb[:, k, :],
                        start=(k == 0), stop=(k == KO - 1),
                    )
                st = stage_pool.tile([P, GBLK], f32)
                nc.vector.tensor_add(st[:], ps[:], b2_t[:])
                nc.sync.dma_start(
                    out=out[tt * P:(tt + 1) * P, :], in_=st[:])
    nc.compile()
    return nc


def _get_nc():
    if "nc" not in _CACHED:
        _CACHED["nc"] = _build_nc()
    return _CACHED["nc"]


def _prep_inputs(value, Wv, bv, Wo, bo):
    import ml_dtypes
    bf16 = ml_dtypes.bfloat16

    x = np.asarray(value, np.float32).reshape(TOK, D)
    Wv = np.asarray(Wv, np.float32)
    Wo = np.asarray(Wo, np.float32)
    bv = np.asarray(bv, np.float32)
    bo = np.asarray(bo, np.float32)

    # xt[tt, p, ko, tc] = x[tt*128+tc, ko*128+p]
    xt = np.ascontiguousarray(
        x.reshape(TT, P, KO, P).transpose(0, 3, 2, 1)).astype(bf16)
    # wv_p[m, p, fo, c2] = Wv[fo*128+p, m*128+c2]
    wv_p = np.ascontiguousarray(
        Wv.reshape(KO, P, KO, P).transpose(2, 1, 0, 3)).astype(bf16)
    # woT_full[c][p, fo, g] = Wo[c*GBLK+g, fo*128+p]
    woT_full = Wo.reshape(N_CORES, GBLK, KO, P).transpose(0, 3, 2, 1)

    bias2 = (Wo.astype(np.float64) @ bv.astype(np.float64)
             + bo.astype(np.float64)).astype(np.float32)

    in_maps = []
    for c in range(N_CORES):
        b2_c = np.ascontiguousarray(np.broadcast_to(
            bias2[c * GBLK:(c + 1) * GBLK][None, :], (P, GBLK)))
        in_maps.append({
            "xt": xt,
            "wv": wv_p,
            "woT": np.ascontiguousarray(woT_full[c]).astype(bf16),
            "b2": b2_c,
        })
    return in_maps


def _run(in_maps, trace=False):
    from concourse.bass_utils import run_bass_kernel_spmd
    nc = _get_nc()
    res = run_bass_kernel_spmd(nc, in_maps, list(range(N_CORES)), trace=trace)
    return res


def kernel(**inputs):
    in_maps = _prep_inputs(inputs["value"], inputs["Wv"], inputs["bv"],
                           inputs["Wo"], inputs["bo"])
    res = _run(in_maps, trace=False)
    out = np.empty((TOK, D), np.float32)
    for c in range(N_CORES):
        out[:, c * GBLK:(c + 1) * GBLK] = res.results[c]["out"]
    return out.reshape(B, S, D)


# revision 27
# speedup vs baseline: 1.0103x; 1.0000x over previous
"""Trainium2 Bass kernel for nn_ExtendedSelfAttention (B=4, S=2048, D=4096, H=1).

With n_heads=1 the softmax is over a size-1 axis, so attention weights are
exactly 1.0 and the module reduces to:

    out = (value @ Wv.T + bv) @ Wo.T + bo
        = value @ (Wo @ Wv).T + (Wo @ bv + bo)

(query/key/Wq/Wk never affect the output.) Since there are 8192 tokens but
only 4096 features, composing the weights first cuts total FLOPs by 25%:
computing Wc^T = (Wo @ Wv)^T costs one 4096^3 GEMM (sharded 8 ways), after
which only ONE token GEMM is needed instead of two.

Sharding (no collectives):
  phase A: core c computes Wc^T[:, c*512:(c+1)*512]   (1024 matmuls)
           lhsT = Wv[f-tile, k-block] (natural layout), rhs = Wo^T slice
  phase B: core c computes out[:, c*512:(c+1)*512] for ALL 8192 tokens
           lhsT = x^T tiles, rhs = Wc^T slice (SBUF-resident)  (2048 matmuls)
Output is column-sharded; the host concatenates. The fused bias
bias2 = Wo @ bv + bo is computed exactly on the host and added in phase B.

Compute dtype bf16 (host-cast), fp32 PSUM accumulation, fp32 output.
"""

import numpy as np

B, S, D = 4, 2048, 4096
N_CORES = 8
TOK = B * S           # 8192 tokens
P = 128
KO = D // P           # 32 contraction tiles
GBLK = D // N_CORES   # 512 output columns per core
TT = TOK // P         # 64 token tiles

_CACHED = {}


def _build_nc():
    import concourse.bass as bass  # noqa: F401  (registers engine builders)
    import concourse.tile as tile
    from concourse import bacc, mybir

    bf16 = mybir.dt.bfloat16
    f32 = mybir.dt.float32

    nc = bacc.Bacc("TRN2", target_bir_lowering=False, debug=False,
                   num_devices=N_CORES)

    # wv[m, p, fo, c2] = Wv[fo*128+p, m*128+c2]   (lhsT tiles for phase A)
    wv = nc.declare_dram_parameter("wv", [KO, P, KO, P], bf16, isOutput=False)
    # woT[p, fo, g] = Wo[cg0+g, fo*128+p]          (rhs for phase A, per-core)
    woT = nc.declare_dram_parameter("woT", [P, KO, GBLK], bf16, isOutput=False)
    # xt[tt, p, ko, tc] = x[tt*128+tc, ko*128+p]   (lhsT tiles for phase B)
    xt = nc.declare_dram_parameter("xt", [TT, P, KO, P], bf16, isOutput=False)
    b2 = nc.declare_dram_parameter("b2", [P, GBLK], f32, isOutput=False)
    out = nc.declare_dram_parameter("out", [TOK, GBLK], f32, isOutput=True)

    with tile.TileContext(nc) as tc:
        with tc.tile_pool(name="const", bufs=1) as const_pool, \
             tc.tile_pool(name="wot", bufs=1) as wot_pool, \
             tc.tile_pool(name="wct", bufs=1) as wct_pool, \
             tc.tile_pool(name="wvp", bufs=4) as wv_pool, \
             tc.tile_pool(name="xtp", bufs=4) as xt_pool, \
             tc.tile_pool(name="psum", bufs=8, space="PSUM") as psum_pool, \
             tc.tile_pool(name="stage", bufs=4) as stage_pool:
            wot_sb = wot_pool.tile([P, KO, GBLK], bf16)
            wct_sb = wct_pool.tile([P, KO, GBLK], bf16)

            # Prewarm the PE during the otherwise-idle DMA ramp (~14us): the
            # HAM clock gate needs ~3.4us of sustained matmul activity to
            # lift the PE from 1.2 to 2.4 GHz, so run one long dummy
            # accumulation group on memset data. Sized to end just before
            # the first real weights land (longer idle re-throttles).
            warm_lhs = const_pool.tile([P, P], bf16, tag="warm_lhs")
            warm_rhs = const_pool.tile([P, GBLK], bf16, tag="warm_rhs")
            nc.vector.memset(warm_lhs[:], 0.0)
            nc.vector.memset(warm_rhs[:], 0.0)
            N_WARM = 40
            dps = psum_pool.tile([P, GBLK], f32, tag="ps")
            for i in range(N_WARM):
                nc.tensor.matmul(dps[:], warm_lhs[:], warm_rhs[:],
                                 start=(i == 0), stop=(i == N_WARM - 1))

            # ---- phase A: Wc^T slice = Wv.T-contracted with Wo^T slice ----
            # Startup: interleave wot 8-ftile chunks (8KB/partition descriptors
            # -> full DMA rate) with the first wv tiles so the first matmul
            # group can start ~5us in and never starves afterwards.
            wv_pre = []
            nc.sync.dma_start(out=wot_sb[:, 0:8, :], in_=woT[:, 0:8, :])
            wv_t = wv_pool.tile([P, KO, P], bf16, tag="wv")
            nc.sync.dma_start(out=wv_t[:], in_=wv[0])
            wv_pre.append(wv_t)
            for g in range(1, 4):
                nc.sync.dma_start(out=wot_sb[:, g * 8:(g + 1) * 8, :],
                                  in_=woT[:, g * 8:(g + 1) * 8, :])
            for m in range(1, 3):
                wv_t = wv_pool.tile([P, KO, P], bf16, tag="wv")
                nc.sync.dma_start(out=wv_t[:], in_=wv[m])
                wv_pre.append(wv_t)

            b2_t = const_pool.tile([P, GBLK], f32)
            nc.sync.dma_start(out=b2_t[:], in_=b2[:])

            for mA in range(KO):
                if mA < 3:
                    wv_t = wv_pre[mA]
                else:
                    wv_t = wv_pool.tile([P, KO, P], bf16, tag="wv")
                    nc.sync.dma_start(out=wv_t[:], in_=wv[mA])
                ps = psum_pool.tile([P, GBLK], f32)
                for fA in range(KO):
                    nc.tensor.matmul(
                        ps[:], wv_t[:, fA, :], wot_sb[:, fA, :],
                        start=(fA == 0), stop=(fA == KO - 1),
                    )
                nc.vector.tensor_copy(wct_sb[:, mA, :], ps[:])

            # ---- phase B: out slice = x @ Wc^T slice (+ bias2) ----
            for tt in range(TT):
                xt_t = xt_pool.tile([P, KO, P], bf16)
                nc.sync.dma_start(out=xt_t[:], in_=xt[tt])
                ps = psum_pool.tile([P, GBLK], f32)
                for k in range(KO):
                    nc.tensor.matmul(
                        ps[:], xt_t[:, k, :], wct_sb[:, k, :],
                        start=(k == 0), stop=(k == KO - 1),
                    )
                st = stage_pool.tile([P, GBLK], f32)
                nc.vector.tensor_add(st[:], ps[:], b2_t[:])
                nc.sync.dma_start(
                    out=out[tt * P:(tt + 1) * P, :], in_=st[:])
    nc.compile()
    return nc


def _get_nc():
    if "nc" not in _CACHED:
        _CACHED["nc"] = _build_nc()
    return _CACHED["nc"]


def _prep_inputs(value, Wv, bv, Wo, bo):
    import ml_dtypes
    bf16 = ml_dtypes.bfloat16

    x = np.asarray(value, np.float32).reshape(TOK, D)
    Wv = np.asarray(Wv, np.float32)
    Wo = np.asarray(Wo, np.float32)
    bv = np.asarray(bv, np.float32)
    bo = np.asarray(bo, np.float32)

    # xt[tt, p, ko, tc] = x[tt*128+tc, ko*128+p]
    xt = np.ascontiguousarray(
        x.reshape(TT, P, KO, P).transpose(0, 3, 2, 1)).astype(bf16)
    # wv_p[m, p, fo, c2] = Wv[fo*128+p, m*128+c2]
    wv_p = np.ascontiguousarray(
        Wv.reshape(KO, P, KO, P).transpose(2, 1, 0, 3)).astype(bf16)
    # woT_full[c][p, fo, g] = Wo[c*GBLK+g, fo*128+p]
    woT_full = Wo.reshape(N_CORES, GBLK, KO, P).transpose(0, 3, 2, 1)

    bias2 = (Wo.astype(np.float64) @ bv.astype(np.float64)
             + bo.astype(np.float64)).astype(np.float32)

    in_maps = []
    for c in range(N_CORES):
        b2_c = np.ascontiguousarray(np.broadcast_to(
            bias2[c * GBLK:(c + 1) * GBLK][None, :], (P, GBLK)))
        in_maps.append({
            "xt": xt,
            "wv": wv_p,
            "woT": np.ascontiguousarray(woT_full[c]).astype(bf16),
            "b2": b2_c,
        })
    return in_maps


def _run(in_maps, trace=False):
    from concourse.bass_utils import run_bass_kernel_spmd
    nc = _get_nc()
    res = run_bass_kernel_spmd(nc, in_maps, list(range(N_CORES)), trace=trace)
    return res


def kernel(**inputs):
    in_maps = _prep_inputs(inputs["value"], inputs["Wv"], inputs["bv"],
                           inputs["Wo"], inputs["bo"])
    res = _run(in_maps, trace=False)
    out = np.empty((TOK, D), np.float32)
    for c in range(N_CORES):
        out[:, c * GBLK:(c + 1) * GBLK] = res.results[c]["out"]
    return out.reshape(B, S, D)


# revision 28
# speedup vs baseline: 1.0130x; 1.0026x over previous
"""Trainium2 Bass kernel for nn_ExtendedSelfAttention (B=4, S=2048, D=4096, H=1).

With n_heads=1 the softmax is over a size-1 axis, so attention weights are
exactly 1.0 and the module reduces to:

    out = (value @ Wv.T + bv) @ Wo.T + bo
        = value @ (Wo @ Wv).T + (Wo @ bv + bo)

(query/key/Wq/Wk never affect the output.) Since there are 8192 tokens but
only 4096 features, composing the weights first cuts total FLOPs by 25%:
computing Wc^T = (Wo @ Wv)^T costs one 4096^3 GEMM (sharded 8 ways), after
which only ONE token GEMM is needed instead of two.

Sharding (no collectives):
  phase A: core c computes Wc^T[:, c*512:(c+1)*512]   (1024 matmuls)
           lhsT = Wv[f-tile, k-block] (natural layout), rhs = Wo^T slice
  phase B: core c computes out[:, c*512:(c+1)*512] for ALL 8192 tokens
           lhsT = x^T tiles, rhs = Wc^T slice (SBUF-resident)  (2048 matmuls)
Output is column-sharded; the host concatenates. The fused bias
bias2 = Wo @ bv + bo is computed exactly on the host and added in phase B.

Compute dtype bf16 (host-cast), fp32 PSUM accumulation, fp32 output.
"""

import numpy as np

B, S, D = 4, 2048, 4096
N_CORES = 8
TOK = B * S           # 8192 tokens
P = 128
KO = D // P           # 32 contraction tiles
GBLK = D // N_CORES   # 512 output columns per core
TT = TOK // P         # 64 token tiles

_CACHED = {}


def _build_nc():
    import concourse.bass as bass  # noqa: F401  (registers engine builders)
    import concourse.tile as tile
    from concourse import bacc, mybir

    bf16 = mybir.dt.bfloat16
    f32 = mybir.dt.float32

    nc = bacc.Bacc("TRN2", target_bir_lowering=False, debug=False,
                   num_devices=N_CORES)

    # wv[m, p, fo, c2] = Wv[fo*128+p, m*128+c2]   (lhsT tiles for phase A)
    wv = nc.declare_dram_parameter("wv", [KO, P, KO, P], bf16, isOutput=False)
    # woT[p, fo, g] = Wo[cg0+g, fo*128+p]          (rhs for phase A, per-core)
    woT = nc.declare_dram_parameter("woT", [P, KO, GBLK], bf16, isOutput=False)
    # xt[tt, p, ko, tc] = x[tt*128+tc, ko*128+p]   (lhsT tiles for phase B)
    xt = nc.declare_dram_parameter("xt", [TT, P, KO, P], bf16, isOutput=False)
    b2 = nc.declare_dram_parameter("b2", [P, GBLK], f32, isOutput=False)
    out = nc.declare_dram_parameter("out", [TOK, GBLK], f32, isOutput=True)

    with tile.TileContext(nc) as tc:
        with tc.tile_pool(name="const", bufs=1) as const_pool, \
             tc.tile_pool(name="wot", bufs=1) as wot_pool, \
             tc.tile_pool(name="wct", bufs=1) as wct_pool, \
             tc.tile_pool(name="wvp", bufs=4) as wv_pool, \
             tc.tile_pool(name="xtp", bufs=4) as xt_pool, \
             tc.tile_pool(name="psum", bufs=8, space="PSUM") as psum_pool, \
             tc.tile_pool(name="stage", bufs=4) as stage_pool:
            wot_sb = wot_pool.tile([P, KO, GBLK], bf16)
            wct_sb = wct_pool.tile([P, KO, GBLK], bf16)

            # Prewarm the PE during the otherwise-idle DMA ramp (~14us): the
            # HAM clock gate needs ~3.4us of sustained matmul activity to
            # lift the PE from 1.2 to 2.4 GHz, so run one long dummy
            # accumulation group on memset data. Sized to end just before
            # the first real weights land (longer idle re-throttles).
            warm_lhs = const_pool.tile([P, P], bf16, tag="warm_lhs")
            warm_rhs = const_pool.tile([P, GBLK], bf16, tag="warm_rhs")
            nc.vector.memset(warm_lhs[:], 0.0)
            nc.vector.memset(warm_rhs[:], 0.0)
            N_WARM = 26
            dps = psum_pool.tile([P, GBLK], f32, tag="ps")
            for i in range(N_WARM):
                nc.tensor.matmul(dps[:], warm_lhs[:], warm_rhs[:],
                                 start=(i == 0), stop=(i == N_WARM - 1))

            # ---- phase A: Wc^T slice = Wv.T-contracted with Wo^T slice ----
            # Startup: interleave wot 8-ftile chunks (8KB/partition descriptors
            # -> full DMA rate) with the first wv tiles so the first matmul
            # group can start ~5us in and never starves afterwards.
            wv_pre = []
            nc.sync.dma_start(out=wot_sb[:, 0:8, :], in_=woT[:, 0:8, :])
            wv_t = wv_pool.tile([P, KO, P], bf16, tag="wv")
            nc.sync.dma_start(out=wv_t[:], in_=wv[0])
            wv_pre.append(wv_t)
            for g in range(1, 4):
                nc.sync.dma_start(out=wot_sb[:, g * 8:(g + 1) * 8, :],
                                  in_=woT[:, g * 8:(g + 1) * 8, :])
            for m in range(1, 3):
                wv_t = wv_pool.tile([P, KO, P], bf16, tag="wv")
                nc.sync.dma_start(out=wv_t[:], in_=wv[m])
                wv_pre.append(wv_t)

            b2_t = const_pool.tile([P, GBLK], f32)
            nc.sync.dma_start(out=b2_t[:], in_=b2[:])

            for mA in range(KO):
                if mA < 3:
                    wv_t = wv_pre[mA]
                else:
                    wv_t = wv_pool.tile([P, KO, P], bf16, tag="wv")
                    nc.sync.dma_start(out=wv_t[:], in_=wv[mA])
                ps = psum_pool.tile([P, GBLK], f32)
                for fA in range(KO):
                    nc.tensor.matmul(
                        ps[:], wv_t[:, fA, :], wot_sb[:, fA, :],
                        start=(fA == 0), stop=(fA == KO - 1),
                    )
                nc.vector.tensor_copy(wct_sb[:, mA, :], ps[:])

            # ---- phase B: out slice = x @ Wc^T slice (+ bias2) ----
            for tt in range(TT):
                xt_t = xt_pool.tile([P, KO, P], bf16)
                nc.sync.dma_start(out=xt_t[:], in_=xt[tt])
                ps = psum_pool.tile([P, GBLK], f32)
                for k in range(KO):
                    nc.tensor.matmul(
                        ps[:], xt_t[:, k, :], wct_sb[:, k, :],
                        start=(k == 0), stop=(k == KO - 1),
                    )
                st = stage_pool.tile([P, GBLK], f32)
                nc.vector.tensor_add(st[:], ps[:], b2_t[:])
                nc.sync.dma_start(
                    out=out[tt * P:(tt + 1) * P, :], in_=st[:])
    nc.compile()
    return nc


def _get_nc():
    if "nc" not in _CACHED:
        _CACHED["nc"] = _build_nc()
    return _CACHED["nc"]


def _prep_inputs(value, Wv, bv, Wo, bo):
    import ml_dtypes
    bf16 = ml_dtypes.bfloat16

    x = np.asarray(value, np.float32).reshape(TOK, D)
    Wv = np.asarray(Wv, np.float32)
    Wo = np.asarray(Wo, np.float32)
    bv = np.asarray(bv, np.float32)
    bo = np.asarray(bo, np.float32)

    # xt[tt, p, ko, tc] = x[tt*128+tc, ko*128+p]
    xt = np.ascontiguousarray(
        x.reshape(TT, P, KO, P).transpose(0, 3, 2, 1)).astype(bf16)
    # wv_p[m, p, fo, c2] = Wv[fo*128+p, m*128+c2]
    wv_p = np.ascontiguousarray(
        Wv.reshape(KO, P, KO, P).transpose(2, 1, 0, 3)).astype(bf16)
    # woT_full[c][p, fo, g] = Wo[c*GBLK+g, fo*128+p]
    woT_full = Wo.reshape(N_CORES, GBLK, KO, P).transpose(0, 3, 2, 1)

    bias2 = (Wo.astype(np.float64) @ bv.astype(np.float64)
             + bo.astype(np.float64)).astype(np.float32)

    in_maps = []
    for c in range(N_CORES):
        b2_c = np.ascontiguousarray(np.broadcast_to(
            bias2[c * GBLK:(c + 1) * GBLK][None, :], (P, GBLK)))
        in_maps.append({
            "xt": xt,
            "wv": wv_p,
            "woT": np.ascontiguousarray(woT_full[c]).astype(bf16),
            "b2": b2_c,
        })
    return in_maps


def _run(in_maps, trace=False):
    from concourse.bass_utils import run_bass_kernel_spmd
    nc = _get_nc()
    res = run_bass_kernel_spmd(nc, in_maps, list(range(N_CORES)), trace=trace)
    return res


def kernel(**inputs):
    in_maps = _prep_inputs(inputs["value"], inputs["Wv"], inputs["bv"],
                           inputs["Wo"], inputs["bo"])
    res = _run(in_maps, trace=False)
    out = np.empty((TOK, D), np.float32)
    for c in range(N_CORES):
        out[:, c * GBLK:(c + 1) * GBLK] = res.results[c]["out"]
    return out.reshape(B, S, D)
